# revision 22
# baseline (speedup 1.0000x reference)
"""CAM (channel attention) kernel for Trainium2, 8-core data-parallel over batch.

Per batch item (one per NeuronCore):
    energy   = Q @ K^T                     (C x C, contract over N)
    att      = softmax(max(energy) - energy) = softmax(-energy)   (shift-invariant)
    out      = gamma * (att @ V) + V

Per-core layout (q,k,v: [C=512, N=4096] f32 in DRAM):
  - q,k streamed in with cast-during-DMA (SWDGE) to bf16 natural chunks, then
    DMA-xbar block-transposes ([128,1024] -> [128,8,128]) into per-n-group
    packed qT/kT tensors, split across both HWDGE rings.
  - energy accumulates in 4 PSUM banks ([128c, 512d]) over 32 n-chunks; the
    rhs spans all four kT c-tiles via a strided 3D access pattern (N=512/MM).
  - softmax over the free dim: DVE min, ACT exp(bias=rowmin, scale=-1) with
    fused row-sum, DVE reciprocal; gamma is folded into the normalization
    scale so matmul2 directly produces gamma*(att@V); att is bf16.
  - att transposed via PE (identity matmul) to attT (bf16).
  - matmul2 (bf16): for each 512-wide output chunk, v slices are cast
    f32->bf16 on DVE/ACT, 4 accumulating matmuls per c-tile, epilogue
    out = psum + v (f32) on DVE, stored per [128,512].
"""

import numpy as np

B, C, H, W = 8, 512, 64, 64
N = H * W  # 4096
P = 128
CT = C // P  # 4 c-tiles
NJ = N // P  # 32 n-chunks
NO = N // 512  # 8 output column chunks

_nc_cache: dict = {}


def _body(nc, tc, cfg):
    from contextlib import ExitStack

    import concourse.mybir as mybir
    from concourse.bass import ts
    from concourse.masks import make_identity

    cfg = cfg or {}
    front = cfg.get("front", "cast_dma")  # cast_dma | f32_cast
    do = lambda phase: phase not in cfg.get("skip", ())

    dt = mybir.dt
    f32, bf16 = dt.float32, dt.bfloat16
    X = mybir.AxisListType.X

    qa = nc.kio["q"].ap().rearrange("(a p) w -> a p w", p=P)
    ka = nc.kio["k"].ap().rearrange("(a p) w -> a p w", p=P)
    va = nc.kio["v"].ap().rearrange("(a p) w -> a p w", p=P)
    ga = nc.kio["gamma"].ap()
    oa = nc.kio["out"].ap().rearrange("(a p) w -> a p w", p=P)
    oa_p = nc.kio["out"].ap().rearrange("(a p) w -> p a w", p=P)

    hwdge = [nc.sync, nc.scalar]

    with ExitStack() as ctx:
        ep = ctx.enter_context

        p_nat = ep(tc.tile_pool(name="nat", bufs=cfg.get("nat_bufs", 6) if not cfg.get("ng") else 3))
        p_T = ep(tc.tile_pool(name="pT", bufs=1))
        p_vf = ep(tc.tile_pool(name="vf", bufs=CT))
        p_att = ep(tc.tile_pool(name="att", bufs=CT))
        p_attT = ep(tc.tile_pool(name="attT", bufs=CT))
        p_small = ep(tc.tile_pool(name="small", bufs=2))
        p_misc = ep(tc.tile_pool(name="misc", bufs=1))
        p_vbs = ep(tc.tile_pool(name="vbs", bufs=2))
        p_es = ep(tc.tile_pool(name="es", bufs=3))

        # gamma broadcast across partitions: [1,1] DRAM -> [128,1] SBUF
        g128 = p_misc.tile([P, 1], f32)
        nc.sync.dma_start(g128[:], ga.broadcast_to([P, 1]))

        ident = p_misc.tile([P, P], bf16)
        make_identity(nc, ident[:])

        # packed transposed tensors, one tile per n-group of GJ chunks:
        # qT[g] is [p, c-tile, jj, 128] with j = g*GJ + jj
        NG = cfg.get("ng", 4)  # n-groups
        GJ = NJ // NG  # chunks per group
        GW = GJ * P  # columns per group chunk (1024)
        qT = [
            p_T.tile([P, CT, GJ, P], bf16, tag=f"qT{g}", name=f"qT{g}")
            for g in range(NG)
        ]
        kT = [
            p_T.tile([P, CT, GJ, P], bf16, tag=f"kT{g}", name=f"kT{g}")
            for g in range(NG)
        ]

        # q,k transposed via DMA xbar (tq='pe' routes q through the PE instead)
        tq = cfg.get("tq", "xbar")  # xbar | pe

        att = []
        v_f = []
        with tc.tile_pool(name="energy", bufs=CT, space="PSUM") as p_energy:
            e_ps = [
                p_energy.tile([P, 512], f32, tag="e", name=f"e{c}")
                for c in range(CT)
            ]

            # stream q,k in [128, GW] chunks (cast-DMA) -> transpose
            if do("loads_qk"):
                with tc.tile_pool(name="ptp", bufs=4, space="PSUM") as p_ptp:
                    for g in range(NG):
                        for c in range(CT):
                            qn = p_nat.tile(
                                [P, GW], bf16, tag="qn", name=f"qn{g}{c}"
                            )
                            nc.gpsimd.dma_start(qn[:], qa[c][:, ts(g, GW)])
                            kn = p_nat.tile(
                                [P, GW], bf16, tag="kn", name=f"kn{g}{c}"
                            )
                            nc.gpsimd.dma_start(kn[:], ka[c][:, ts(g, GW)])
                            if not do("tpose"):
                                continue
                            if tq == "pe":
                                for jj in range(GJ):
                                    ptp = p_ptp.tile([P, P], bf16)
                                    nc.tensor.transpose(
                                        ptp[:], qn[:, ts(jj, P)], ident[:]
                                    )
                                    if (c * GJ + jj) % 2 == 0:
                                        nc.vector.tensor_copy(
                                            qT[g][:, c, jj, :], ptp[:]
                                        )
                                    else:
                                        nc.scalar.copy(
                                            qT[g][:, c, jj, :], ptp[:]
                                        )
                            else:
                                hwdge[c % 2].dma_start(
                                    qT[g][:, c], qn[:], transpose=True
                                )
                            hwdge[(c + 1) % 2].dma_start(
                                kT[g][:, c], kn[:], transpose=True
                            )

            # v loads (f32)
            if do("loads_v"):
                v_eng = cfg.get("v_eng", "hwdge")
                for c in range(CT):
                    vf = p_vf.tile([P, N], f32, tag="vf", name=f"vf{c}")
                    if v_eng == "swdge":
                        nc.gpsimd.dma_start(vf[:], va[c])
                    else:
                        hwdge[c % 2].dma_start(vf[:], va[c])
                    v_f.append(vf)

            if not (do("loads_qk") and do("tpose") and do("mm1")):
                return

            # matmul1: energy[c] += qT[:,c,j,:].T @ kT[:,:,j,:]  (N=512)
            for g in range(NG):
                for jj in range(GJ):
                    for c in range(CT):
                        nc.tensor.matmul(
                            e_ps[c][:],
                            qT[g][:, c, jj, :],
                            kT[g][:, :, jj, :],
                            start=(g == 0 and jj == 0),
                            stop=(g == NG - 1 and jj == GJ - 1),
                        )

            # softmax(-energy) rows, gamma folded into the normalization
            for c in range(CT):
                rowmin = p_small.tile([P, 1], f32)
                nc.vector.tensor_reduce(
                    rowmin[:], e_ps[c][:], axis=X, op=mybir.AluOpType.min
                )
                pexp = p_att.tile([P, 512], bf16, tag="att", name=f"att{c}")
                rowsum = p_small.tile([P, 1], f32)
                nc.scalar.activation(
                    pexp[:],
                    e_ps[c][:],
                    mybir.ActivationFunctionType.Exp,
                    bias=rowmin[:, 0:1],
                    scale=-1.0,
                    accum_out=rowsum[:, 0:1],
                )
                recip = p_small.tile([P, 1], f32)
                nc.vector.reciprocal(recip[:], rowsum[:])
                srow = p_small.tile([P, 1], f32)
                nc.vector.tensor_scalar_mul(srow[:], recip[:], g128[:, 0:1])
                nc.vector.tensor_scalar_mul(pexp[:], pexp[:], srow[:, 0:1])
                att.append(pexp)

        if not do("mm2"):
            return

        # transpose att (bf16) via PE into attT[d][:, c-block]
        attT = []
        with tc.tile_pool(name="pst", bufs=2, space="PSUM") as p_pst:
            for d in range(CT):
                at = p_attT.tile([P, C], bf16, tag="attT", name=f"attT{d}")
                for c in range(CT):
                    pst = p_pst.tile([P, P], bf16)
                    nc.tensor.transpose(pst[:], att[c][:, ts(d, P)], ident[:])
                    nc.vector.tensor_copy(at[:, ts(c, P)], pst[:])
                attT.append(at)

        # matmul2 (bf16): psum = gamma*(att @ V); epilogue adds v (f32)
        with tc.tile_pool(name="ps2", bufs=3, space="PSUM") as p_ps2:
            for no in range(NO):
                vbs = []
                for d in range(CT):
                    vb = p_vbs.tile(
                        [P, 512], bf16, tag=f"vb{d}", name=f"vb{d}_{no}"
                    )
                    if d % 2 == 0:
                        nc.vector.tensor_copy(vb[:], v_f[d][:, ts(no, 512)])
                    else:
                        nc.scalar.copy(vb[:], v_f[d][:, ts(no, 512)])
                    vbs.append(vb)
                es4 = p_es.tile([P, CT, 512], f32)
                for c in range(CT):
                    ps2 = p_ps2.tile([P, 512], f32)
                    for d in range(CT):
                        nc.tensor.matmul(
                            ps2[:],
                            attT[d][:, ts(c, P)],
                            vbs[d][:],
                            start=(d == 0),
                            stop=(d == CT - 1),
                        )
                    nc.vector.tensor_add(
                        es4[:, c, :], ps2[:], v_f[c][:, ts(no, 512)]
                    )
                hwdge[no % 2].dma_start(oa_p[:, :, ts(no, 512)], es4[:])


def build(repeat=1, cfg=None, loop_n=None):
    import concourse.mybir as mybir
    import concourse.tile as tile
    from concourse import bacc

    dt = mybir.dt
    nc = bacc.Bacc("TRN2", target_bir_lowering=False, debug=False)
    nc.kio = {}
    for name in ("q", "k", "v"):
        nc.kio[name] = nc.dram_tensor(
            name, [C, N], dt.float32, kind="ExternalInput"
        )
    nc.kio["gamma"] = nc.dram_tensor(
        "gamma", [1, 1], dt.float32, kind="ExternalInput"
    )
    nc.kio["out"] = nc.dram_tensor(
        "out", [C, N], dt.float32, kind="ExternalOutput"
    )
    with tile.TileContext(nc) as tc:
        if loop_n is not None:
            with tc.For_i(0, loop_n, 1):
                _body(nc, tc, cfg)
        else:
            for _ in range(repeat):
                _body(nc, tc, cfg)
    nc.compile()
    return nc


def _get_nc():
    if "nc" not in _nc_cache:
        _nc_cache["nc"] = build(repeat=1)
    return _nc_cache["nc"]


def make_in_maps(q, k, v, gamma):
    q = np.ascontiguousarray(np.asarray(q, dtype=np.float32).reshape(B, C, N))
    k = np.ascontiguousarray(np.asarray(k, dtype=np.float32).reshape(B, C, N))
    v = np.ascontiguousarray(np.asarray(v, dtype=np.float32).reshape(B, C, N))
    g = np.asarray(gamma, dtype=np.float32).reshape(1, 1)
    return [
        {"q": q[i], "k": k[i], "v": v[i], "gamma": g} for i in range(B)
    ]


def kernel(q, k, v, gamma):
    from concourse import bass_utils

    nc = _get_nc()
    in_maps = make_in_maps(q, k, v, gamma)
    res = bass_utils.run_bass_kernel_spmd(nc, in_maps, core_ids=list(range(B)))
    out = np.stack([res.results[i]["out"] for i in range(B)])
    return out.reshape(B, C, H, W).astype(np.float32, copy=False)
